# revision 3
# baseline (speedup 1.0000x reference)
"""Causal scaled-dot-product attention on 8 NeuronCores (Trainium2, Bass/Tile).

Same algebra as the first working version (M = Wq Wk^T folding, fp8
DoubleRow logit chain, bf16 V/U chain), restructured for the measured PE
cost curve:

  - All matmuls stream <=256-column chunks: measured ~0.41 ns/col vs
    ~0.47 ns/col at N=512 (microbench: dr256 104 ns/MM vs dr512 244 ns/MM;
    bf256 ~95 vs bf512 ~210+). Two 256-chunk chains fill each 512-wide
    PSUM tile, so the 512-wide ACT/DVE ops are unchanged. The chunk
    chains within one PSUM tile must run sequentially: start=True zeroes
    the whole 2KB zero-region (bank), so interleaving two accumulation
    chains in one bank drops the first chain's first contribution.
  - bv is folded into V at projection time (softmax rows sum to 1, so
    attn @ (V + bv) = attn@V + bv up to ~1e-3 relative Z rounding on bv):
    kills the per-output-tile bias add on DVE.
  - A-proj bias + fp8 cast moved to ACT (activation Identity with
    per-partition bias AP); DVE keeps es accumulation, mask, V add-copy,
    u_out scaling. Balance: PE ~110us > DVE ~30us, ACT ~40us.
  - causal mask multiply only on the 128-wide diagonal band (the rest of
    a diagonal k-tile's q-range is fully visible).
  - psAcc has 3 bufs (tail U tiles + next-iteration proj overlap); the
    z-row and z-transpose matmuls share one PSUM bank (zt tile),
    temporally separated by the DVE copy of the z row.

Measurement note: this box has co-tenant load that inflates device time
30%+ in bad windows; only same-session interleaved comparisons are
meaningful. Under matched load this kernel measured ~12-15% faster than
the 512-wide predecessor (e.g. 178-189us vs 182-227us min-slopes in one
loaded session; ~120us vs ~141us in a quieter one).

Algebra recap (see v1 git history): with M = Wq Wk^T, wc = Wk bq,
  softmax_k(QK^T/32) = softmax_k((x (32M) x^T)/32 + c); the c bias is the
  +32wc[e] row add on A^T. E = exp(scores/1024) bf16 (ACT, one op per
  k-tile); Z via ones-matmul; out = (E^T V')/Z with V' = xWv + bv.

reps>1 wraps the body in a hardware For_i loop for slope timing.
"""

import numpy as np
import ml_dtypes

B = 8
S = 2048
D = 1024
O = 512
P = 128
N_CORES = 8

_CACHE = {}


def _build_nc(s=S, reps=1, loop_phase="all"):
    from contextlib import ExitStack

    import concourse.tile as tile
    import concourse.mybir as mybir
    from concourse import bacc
    from concourse.bass import ds, ts

    assert loop_phase in ("all", "proj"), loop_phase

    f32 = mybir.dt.float32
    bf16 = mybir.dt.bfloat16
    f8 = mybir.dt.float8e4
    AF = mybir.ActivationFunctionType
    DR = mybir.MatmulPerfMode.DoubleRow

    DO = D // P            # 8 d-tiles
    EO = D // P            # 8 e-tiles
    HO = DO // 2           # 4 DoubleRow d-tile pairs
    QBLK = 512             # q-block (macro phase granularity)
    CW = 256               # matmul chunk width (moving free dim)
    NQB = s // QBLK        # q-blocks
    NKT = s // P           # k-tiles

    nc = bacc.Bacc(None, target_bir_lowering=False, debug=False)

    xT = nc.dram_tensor("xT", (D, s), bf16, kind="ExternalInput")
    x8_d = nc.dram_tensor("x8", (D, s), f8, kind="ExternalInput")
    m8_d = nc.dram_tensor("m8", (D, D), f8, kind="ExternalInput")
    wv = nc.dram_tensor("wv", (D, O), bf16, kind="ExternalInput")
    wcp_d = nc.dram_tensor("wcp", (P, EO), f32, kind="ExternalInput")
    bv_rep = nc.dram_tensor("bv_rep", (P, O), f32, kind="ExternalInput")
    mask = nc.dram_tensor("mask", (4, P, QBLK), bf16, kind="ExternalInput")
    out = nc.dram_tensor("out", (s, O), f32, kind="ExternalOutput")

    with tile.TileContext(nc) as tc, ExitStack() as ctx:
        persist = ctx.enter_context(tc.tile_pool(name="persist", bufs=1))
        apool = ctx.enter_context(tc.tile_pool(name="apool", bufs=2))
        vpool = ctx.enter_context(tc.tile_pool(name="vpool", bufs=2))
        etp = ctx.enter_context(tc.tile_pool(name="et", bufs=2))
        esp = ctx.enter_context(tc.tile_pool(name="esp", bufs=2))
        small = ctx.enter_context(tc.tile_pool(name="small", bufs=4))
        outp = ctx.enter_context(tc.tile_pool(name="outp", bufs=3))
        psAcc = ctx.enter_context(tc.tile_pool(name="psAcc", bufs=3, space="PSUM"))
        psS = ctx.enter_context(tc.tile_pool(name="psS", bufs=4, space="PSUM"))
        psZT = ctx.enter_context(tc.tile_pool(name="psZT", bufs=1, space="PSUM"))

        x8_sb = persist.tile([P, DO, s], f8)      # x^T fp8 (logit chain)
        m_sb = persist.tile([P, DO, D], f8)       # 32*(Wq Wk^T) fp8
        xT_sb = persist.tile([P, DO, s], bf16)    # x^T bf16 (V-proj)
        wv_sb = persist.tile([P, DO, O], bf16)
        wcp_sb = persist.tile([P, EO], f32)       # 32*(Wk bq), e-tile-major
        mask_sb = persist.tile([P, 4, QBLK], bf16)
        bv_sb = persist.tile([P, O], f32)
        ones_sb = persist.tile([P, 1], bf16)
        onef_sb = persist.tile([1, 1], f32)

        m_r = m8_d.rearrange("(do p) e -> p do e", p=P)
        x8_r = x8_d.rearrange("(do p) s -> p do s", p=P)
        xT_r = xT.rearrange("(do p) s -> p do s", p=P)
        wv_r = wv.rearrange("(do p) o -> p do o", p=P)
        for do in range(DO):
            nc.sync.dma_start(m_sb[:, do], m_r[:, do])
            nc.sync.dma_start(x8_sb[:, do], x8_r[:, do])
        nc.sync.dma_start(wcp_sb[:], wcp_d[:])
        for do in range(DO):
            nc.sync.dma_start(xT_sb[:, do], xT_r[:, do])
            nc.sync.dma_start(wv_sb[:, do], wv_r[:, do])
        nc.sync.dma_start(mask_sb[:], mask.rearrange("m p q -> p m q"))
        nc.sync.dma_start(bv_sb[:], bv_rep[:])
        nc.vector.memset(ones_sb[:], 1.0)
        nc.vector.memset(onef_sb[:], 1.0)

        def proj(sb, v_sb, st):
            a_t = apool.tile([P, EO, QBLK], f8, name="a_t")
            st["a", sb] = a_t
            for eo in range(EO):
                ps = psAcc.tile([P, QBLK], f32, tag="acc", name="ps_a")
                for c in range(QBLK // CW):
                    for h in range(HO):
                        nc.tensor.matmul(
                            ps[:, ds(CW * c, CW)],
                            lhsT=m_sb[:, 2 * h : 2 * h + 2, ts(eo, P)],
                            rhs=x8_sb[:, 2 * h : 2 * h + 2, ds(QBLK * sb + CW * c, CW)],
                            start=(h == 0), stop=(h == HO - 1), perf_mode=DR,
                        )
                # fused c-bias + fp8 cast on ACT: A'[e,q] = A[e,q] + 32*wc[e]
                nc.scalar.activation(
                    out=a_t[:, eo, :], in_=ps[:], func=AF.Identity,
                    bias=wcp_sb[:, eo : eo + 1],
                )
            for stt in range(QBLK // P):
                ps = psAcc.tile([P, QBLK], f32, tag="acc", name="ps_v")
                for c in range(O // CW):
                    for do in range(DO):
                        nc.tensor.matmul(
                            ps[:, ds(CW * c, CW)],
                            lhsT=xT_sb[:, do, ds(QBLK * sb + P * stt, P)],
                            rhs=wv_sb[:, do, ds(CW * c, CW)],
                            start=(do == 0), stop=(do == DO - 1),
                        )
                # V' = xWv + bv (bv folded into V: attn rows sum to 1)
                nc.vector.tensor_add(
                    v_sb[:, sb * (QBLK // P) + stt, :], ps[:, :O], bv_sb[:]
                )

        def sc_chain(qb, kt, et, es, a_t):
            # diagonal k-tile m covers q >= 128*m; only the 128-wide band
            # [128m, 128m+128) needs the causal mask multiply.
            m = kt - 4 * qb
            q0 = max(m, 0) * P
            qw = QBLK - q0
            ps = psS.tile([P, QBLK], f32, tag="sc", name="ps_s")
            for cs in range(0, QBLK, CW):
                lo = max(q0, cs)
                w = cs + CW - lo
                if w <= 0:
                    continue
                for h in range(HO):
                    nc.tensor.matmul(
                        ps[:, ds(lo, w)],
                        lhsT=x8_sb[:, 2 * h : 2 * h + 2, ts(kt, P)],
                        rhs=a_t[:, 2 * h : 2 * h + 2, ds(lo, w)],
                        start=(h == 0), stop=(h == HO - 1), perf_mode=DR,
                    )
            nc.scalar.activation(
                out=et[:, kt, q0:], in_=ps[:, ds(q0, qw)], func=AF.Exp,
                scale=1.0 / 1024.0,
            )
            if m >= 0:
                nc.vector.tensor_mul(
                    et[:, kt, ds(q0, P)], et[:, kt, ds(q0, P)],
                    mask_sb[:, m, ds(q0, P)],
                )
            if kt == 0:
                nc.vector.tensor_copy(es[:], et[:, 0, :])
            else:
                nc.vector.tensor_add(es[:, q0:], es[:, q0:], et[:, kt, q0:])

        def scores(qb, st, inter=()):
            # U chains of the previous block run between score chains so the
            # PE stays busy while ACT drains the exp backlog.
            nkt = 4 * qb + 4
            et = etp.tile([P, nkt, QBLK], bf16, name="et")
            es = esp.tile([P, QBLK], bf16, name="es")
            st["et", qb] = et
            st["es", qb] = es
            a_t = st["a", qb]
            inter = list(inter)
            for kt in range(nkt):
                sc_chain(qb, kt, et, es, a_t)
                if kt % 4 == 3 and inter:
                    inter.pop(0)()
            for fn in inter:
                fn()

        def finish_z(qb, st):
            # one PSUM bank serves both z stages: the ones-matmul row lives in
            # row 0 of zt; after the DVE copies it out, the 4 transpose
            # matmuls reuse columns 0..3 of the same bank.
            es = st["es", qb]
            zt = psZT.tile([P, QBLK], f32, tag="zt", name="zt")
            nc.tensor.matmul(
                zt[0:1, :], lhsT=ones_sb[:], rhs=es[:], start=True, stop=True,
                skip_group_check=True,
            )
            z_sb = small.tile([1, QBLK], f32, name="z_sb")
            nc.vector.tensor_copy(z_sb[:], zt[0:1, :])
            for j in range(QBLK // P):
                nc.tensor.matmul(
                    zt[:, j : j + 1], lhsT=z_sb[:, ts(j, P)], rhs=onef_sb[:],
                    start=True, stop=True, skip_group_check=True,
                )
            r_sb = small.tile([P, 4], f32, name="r_sb")
            nc.vector.reciprocal(r_sb[:], zt[:, 0:4])
            st["r", qb] = r_sb

        def u_chain(qb, j, v_sb, st, emit_out=True):
            et = st["et", qb]
            qs = qb * (QBLK // P) + j
            ups = psAcc.tile([P, QBLK], f32, tag="acc", name="ups")
            # chunk-major: start=True zeroes the whole 2KB zero-region, so the
            # two 256-wide accumulation chains must not interleave.
            for c in range(O // CW):
                for kt in range(qs + 1):
                    nc.tensor.matmul(
                        ups[:, ds(CW * c, CW)], lhsT=et[:, kt, ts(j, P)],
                        rhs=v_sb[:, kt, ds(CW * c, CW)],
                        start=(kt == 0), stop=(kt == qs),
                    )
            st["ups", qb, j] = ups
            if emit_out:
                u_out(qb, j, st)

        def u_out(qb, j, st):
            qs = qb * (QBLK // P) + j
            ups = st["ups", qb, j]
            r_sb = st["r", qb]
            o_sb = outp.tile([P, O], f32, name="o_sb")
            nc.vector.tensor_scalar_mul(o_sb[:], ups[:, :O], r_sb[:, j : j + 1])
            nc.sync.dma_start(out[ds(P * qs, P), :], o_sb[:])

        def body(_iv=None):
            st = {}
            v_sb = vpool.tile([P, NKT, O], bf16, name="v_sb")
            for sb in range(NQB):
                proj(sb, v_sb, st)
                if loop_phase == "proj":
                    continue
                inter = ()
                if sb >= 1:
                    finish_z(sb - 1, st)
                    inter = [
                        (lambda j: lambda: u_chain(sb - 1, j, v_sb, st))(j)
                        for j in range(QBLK // P)
                    ]
                scores(sb, st, inter)
            if loop_phase != "proj":
                # tail: U(3) j=0,1 first (need only already-exp'd k-tiles),
                # z once the ACT backlog drains, then the rest.
                qb = NQB - 1
                u_chain(qb, 0, v_sb, st, emit_out=False)
                u_chain(qb, 1, v_sb, st, emit_out=False)
                finish_z(qb, st)
                u_out(qb, 0, st)
                u_out(qb, 1, st)
                u_chain(qb, 2, v_sb, st)
                u_chain(qb, 3, v_sb, st)

        if reps == 1:
            body()
        else:
            with tc.For_i(0, reps, 1, hint_engines=(mybir.EngineType.PE,)) as iv:
                body(iv)

    nc.compile()
    return nc


def _get_nc(s=S, reps=1, loop_phase="all"):
    key = (s, reps, loop_phase)
    if key not in _CACHE:
        _CACHE[key] = _build_nc(s, reps, loop_phase)
    return _CACHE[key]


def make_mask(qblk=512):
    kp = np.arange(P)[:, None]
    qf = np.arange(qblk)[None, :]
    m = np.stack([(qf >= P * i + kp) for i in range(4)], axis=0)
    return m.astype(ml_dtypes.bfloat16)


def make_in_maps(x, Wq, bq, Wk, bk, Wv, bv, s=S):
    bf = ml_dtypes.bfloat16
    f8 = ml_dtypes.float8_e4m3
    x, Wq, bq, Wk, bk, Wv, bv = (
        np.asarray(a, dtype=np.float32) for a in (x, Wq, bq, Wk, bk, Wv, bv)
    )
    M = (Wq.astype(np.float64) @ Wk.T.astype(np.float64)).astype(np.float32)
    wc = (Wk @ bq).astype(np.float32)
    m8 = np.ascontiguousarray(np.clip(32.0 * M, -240, 240).astype(f8))
    wv_b = np.ascontiguousarray(Wv.astype(bf))
    wcp = np.ascontiguousarray((32.0 * wc).reshape(D // P, P).T.astype(np.float32))
    bv_rep = np.ascontiguousarray(np.broadcast_to(bv, (P, O)))
    mask = make_mask()
    in_maps = []
    for b in range(x.shape[0]):
        xT_b = np.ascontiguousarray(x[b].T.astype(bf))
        x8_b = np.ascontiguousarray(np.clip(x[b].T, -240, 240).astype(f8))
        in_maps.append(
            dict(xT=xT_b, x8=x8_b, m8=m8, wv=wv_b, wcp=wcp, bv_rep=bv_rep, mask=mask)
        )
    return in_maps


def kernel(x, Wq, bq, Wk, bk, Wv, bv):
    from concourse.bass_utils import run_bass_kernel_spmd

    x = np.asarray(x, dtype=np.float32)
    assert x.shape == (B, S, D), x.shape
    nc = _get_nc(S)
    in_maps = make_in_maps(x, Wq, bq, Wk, bk, Wv, bv)
    res = run_bass_kernel_spmd(nc, in_maps, core_ids=list(range(N_CORES)))
    return np.stack([res.results[c]["out"] for c in range(N_CORES)], axis=0)


# revision 5
# speedup vs baseline: 1.0413x; 1.0413x over previous
"""Causal scaled-dot-product attention on 8 NeuronCores (Trainium2, Bass/Tile).

Same algebra as the original working kernel (M = Wq Wk^T folding, fp8
DoubleRow logit chain, bf16 V/U chain), restructured for the measured PE
cost curve:

  - All matmuls stream <=256-column chunks: measured ~0.41 ns/col vs
    ~0.47 ns/col at N=512 (micro3: dr256 104 ns/MM vs dr512 244 ns/MM;
    bf256 ~95 vs bf512 ~210+).
  - fp8-DR chunk chains each get their own PSUM bank (psD pool): two DR
    chains sharing a bank cost ~+20 ns/MM (micro4). bf16 chains show no
    such penalty; V/U keep 512-wide PSUM tiles (psB) filled by two
    sequential 256-chunk chains (start=True zeroes the whole 2KB
    zero-region, so chains within a bank must not interleave).
  - bv is folded into V at projection time (softmax rows sum to 1, so
    attn @ (V + bv) = attn@V + bv up to ~1e-3 relative Z rounding on bv):
    kills the per-output-tile bias add on DVE.
  - A-proj bias + fp8 cast moved to ACT (activation Identity with
    per-partition bias AP); DVE keeps es accumulation, mask, V add-copy,
    u_out scaling. Balance: PE ~110us > DVE ~30us, ACT ~40us.
  - causal mask multiply only on the 128-wide diagonal band (the rest of
    a diagonal k-tile's q-range is fully visible).

Algebra recap (see v1 git history): with M = Wq Wk^T, wc = Wk bq,
  softmax_k(QK^T/32) = softmax_k((x (32M) x^T)/32 + c); the c bias is the
  +32wc[e] row add on A^T. E = exp(scores/1024) bf16 (ACT, one op per
  k-tile); Z via ones-matmul; out = (E^T V')/Z with V' = xWv + bv.

reps>1 wraps the body in a hardware For_i loop for slope timing.
"""

import numpy as np
import ml_dtypes

B = 8
S = 2048
D = 1024
O = 512
P = 128
N_CORES = 8

_CACHE = {}


def _build_nc(s=S, reps=1, loop_phase="all"):
    from contextlib import ExitStack

    import concourse.tile as tile
    import concourse.mybir as mybir
    from concourse import bacc
    from concourse.bass import ds, ts

    assert loop_phase in ("all", "proj"), loop_phase

    f32 = mybir.dt.float32
    bf16 = mybir.dt.bfloat16
    f8 = mybir.dt.float8e4
    AF = mybir.ActivationFunctionType
    DR = mybir.MatmulPerfMode.DoubleRow

    DO = D // P            # 8 d-tiles
    EO = D // P            # 8 e-tiles
    HO = DO // 2           # 4 DoubleRow d-tile pairs
    QBLK = 512             # q-block (macro phase granularity)
    CW = 256               # matmul chunk width (moving free dim)
    NQB = s // QBLK        # q-blocks
    NKT = s // P           # k-tiles

    nc = bacc.Bacc(None, target_bir_lowering=False, debug=False)

    xT = nc.dram_tensor("xT", (D, s), bf16, kind="ExternalInput")
    x8_d = nc.dram_tensor("x8", (D, s), f8, kind="ExternalInput")
    m8_d = nc.dram_tensor("m8", (D, D), f8, kind="ExternalInput")
    wv = nc.dram_tensor("wv", (D, O), bf16, kind="ExternalInput")
    wcp_d = nc.dram_tensor("wcp", (P, EO), f32, kind="ExternalInput")
    bv_rep = nc.dram_tensor("bv_rep", (P, O), f32, kind="ExternalInput")
    mask = nc.dram_tensor("mask", (4, P, QBLK), bf16, kind="ExternalInput")
    out = nc.dram_tensor("out", (s, O), f32, kind="ExternalOutput")

    with tile.TileContext(nc) as tc, ExitStack() as ctx:
        persist = ctx.enter_context(tc.tile_pool(name="persist", bufs=1))
        apool = ctx.enter_context(tc.tile_pool(name="apool", bufs=2))
        vpool = ctx.enter_context(tc.tile_pool(name="vpool", bufs=2))
        etp = ctx.enter_context(tc.tile_pool(name="et", bufs=2))
        esp = ctx.enter_context(tc.tile_pool(name="esp", bufs=2))
        small = ctx.enter_context(tc.tile_pool(name="small", bufs=4))
        outp = ctx.enter_context(tc.tile_pool(name="outp", bufs=3))
        # fp8-DR chains into a shared bank cost ~+20ns/MM vs own-bank chains
        # (micro4); DR chunk chains get per-chunk banks from psD. bf16 chains
        # show no such penalty and stay 512-wide in psB.
        psD = ctx.enter_context(tc.tile_pool(name="psD", bufs=4, space="PSUM"))
        psB = ctx.enter_context(tc.tile_pool(name="psB", bufs=3, space="PSUM"))
        psZT = ctx.enter_context(tc.tile_pool(name="psZT", bufs=1, space="PSUM"))

        x8_sb = persist.tile([P, DO, s], f8)      # x^T fp8 (logit chain)
        m_sb = persist.tile([P, DO, D], f8)       # 32*(Wq Wk^T) fp8
        xT_sb = persist.tile([P, DO, s], bf16)    # x^T bf16 (V-proj)
        wv_sb = persist.tile([P, DO, O], bf16)
        wcp_sb = persist.tile([P, EO], f32)       # 32*(Wk bq), e-tile-major
        mask_sb = persist.tile([P, 4, QBLK], bf16)
        bv_sb = persist.tile([P, O], f32)
        ones_sb = persist.tile([P, 1], bf16)
        onef_sb = persist.tile([1, 1], f32)

        m_r = m8_d.rearrange("(do p) e -> p do e", p=P)
        x8_r = x8_d.rearrange("(do p) s -> p do s", p=P)
        xT_r = xT.rearrange("(do p) s -> p do s", p=P)
        wv_r = wv.rearrange("(do p) o -> p do o", p=P)
        for do in range(DO):
            nc.sync.dma_start(m_sb[:, do], m_r[:, do])
            nc.sync.dma_start(x8_sb[:, do], x8_r[:, do])
        nc.sync.dma_start(wcp_sb[:], wcp_d[:])
        for do in range(DO):
            nc.sync.dma_start(xT_sb[:, do], xT_r[:, do])
            nc.sync.dma_start(wv_sb[:, do], wv_r[:, do])
        nc.sync.dma_start(mask_sb[:], mask.rearrange("m p q -> p m q"))
        nc.sync.dma_start(bv_sb[:], bv_rep[:])
        nc.vector.memset(ones_sb[:], 1.0)
        nc.vector.memset(onef_sb[:], 1.0)

        def proj(sb, v_sb, st):
            a_t = apool.tile([P, EO, QBLK], f8, name="a_t")
            st["a", sb] = a_t
            for eo in range(EO):
                for c in range(QBLK // CW):
                    ps = psD.tile([P, CW], f32, tag="dr", name="ps_a")
                    for h in range(HO):
                        nc.tensor.matmul(
                            ps[:],
                            lhsT=m_sb[:, 2 * h : 2 * h + 2, ts(eo, P)],
                            rhs=x8_sb[:, 2 * h : 2 * h + 2, ds(QBLK * sb + CW * c, CW)],
                            start=(h == 0), stop=(h == HO - 1), perf_mode=DR,
                        )
                    # fused c-bias + fp8 cast on ACT: A' = A + 32*wc[e]
                    nc.scalar.activation(
                        out=a_t[:, eo, ds(CW * c, CW)], in_=ps[:], func=AF.Identity,
                        bias=wcp_sb[:, eo : eo + 1],
                    )
            for stt in range(QBLK // P):
                ps = psB.tile([P, QBLK], f32, tag="acc", name="ps_v")
                for c in range(O // CW):
                    for do in range(DO):
                        nc.tensor.matmul(
                            ps[:, ds(CW * c, CW)],
                            lhsT=xT_sb[:, do, ds(QBLK * sb + P * stt, P)],
                            rhs=wv_sb[:, do, ds(CW * c, CW)],
                            start=(do == 0), stop=(do == DO - 1),
                        )
                # V' = xWv + bv (bv folded into V: attn rows sum to 1)
                nc.vector.tensor_add(
                    v_sb[:, sb * (QBLK // P) + stt, :], ps[:, :O], bv_sb[:]
                )

        def sc_chain(qb, kt, et, es, a_t):
            # diagonal k-tile m covers q >= 128*m; only the 128-wide band
            # [128m, 128m+128) needs the causal mask multiply.
            m = kt - 4 * qb
            q0 = max(m, 0) * P
            for cs in range(0, QBLK, CW):
                lo = max(q0, cs)
                w = cs + CW - lo
                if w <= 0:
                    continue
                ps = psD.tile([P, CW], f32, tag="dr", name="ps_s")
                for h in range(HO):
                    nc.tensor.matmul(
                        ps[:, :w],
                        lhsT=x8_sb[:, 2 * h : 2 * h + 2, ts(kt, P)],
                        rhs=a_t[:, 2 * h : 2 * h + 2, ds(lo, w)],
                        start=(h == 0), stop=(h == HO - 1), perf_mode=DR,
                    )
                nc.scalar.activation(
                    out=et[:, kt, ds(lo, w)], in_=ps[:, :w], func=AF.Exp,
                    scale=1.0 / 1024.0,
                )
                if m >= 0 and lo <= P * m < cs + CW:
                    nc.vector.tensor_mul(
                        et[:, kt, ds(P * m, P)], et[:, kt, ds(P * m, P)],
                        mask_sb[:, m, ds(P * m, P)],
                    )
                if kt == 0:
                    nc.vector.tensor_copy(es[:, ds(lo, w)], et[:, kt, ds(lo, w)])
                else:
                    nc.vector.tensor_add(
                        es[:, ds(lo, w)], es[:, ds(lo, w)], et[:, kt, ds(lo, w)]
                    )

        def scores(qb, st, inter=()):
            # U chains of the previous block run between score chains so the
            # PE stays busy while ACT drains the exp backlog.
            nkt = 4 * qb + 4
            et = etp.tile([P, nkt, QBLK], bf16, name="et")
            es = esp.tile([P, QBLK], bf16, name="es")
            st["et", qb] = et
            st["es", qb] = es
            a_t = st["a", qb]
            inter = list(inter)
            for kt in range(nkt):
                sc_chain(qb, kt, et, es, a_t)
                if kt % 4 == 3 and inter:
                    inter.pop(0)()
            for fn in inter:
                fn()

        def finish_z(qb, st):
            # one PSUM bank serves both z stages: the ones-matmul row lives in
            # row 0 of zt; after the DVE copies it out, the 4 transpose
            # matmuls reuse columns 0..3 of the same bank.
            es = st["es", qb]
            zt = psZT.tile([P, QBLK], f32, tag="zt", name="zt")
            nc.tensor.matmul(
                zt[0:1, :], lhsT=ones_sb[:], rhs=es[:], start=True, stop=True,
                skip_group_check=True,
            )
            z_sb = small.tile([1, QBLK], f32, name="z_sb")
            nc.vector.tensor_copy(z_sb[:], zt[0:1, :])
            for j in range(QBLK // P):
                nc.tensor.matmul(
                    zt[:, j : j + 1], lhsT=z_sb[:, ts(j, P)], rhs=onef_sb[:],
                    start=True, stop=True, skip_group_check=True,
                )
            r_sb = small.tile([P, 4], f32, name="r_sb")
            nc.vector.reciprocal(r_sb[:], zt[:, 0:4])
            st["r", qb] = r_sb

        def u_chain(qb, j, v_sb, st, emit_out=True):
            et = st["et", qb]
            qs = qb * (QBLK // P) + j
            ups = psB.tile([P, QBLK], f32, tag="acc", name="ups")
            # chunk-major: start=True zeroes the whole 2KB zero-region, so the
            # two 256-wide accumulation chains must not interleave.
            for c in range(O // CW):
                for kt in range(qs + 1):
                    nc.tensor.matmul(
                        ups[:, ds(CW * c, CW)], lhsT=et[:, kt, ts(j, P)],
                        rhs=v_sb[:, kt, ds(CW * c, CW)],
                        start=(kt == 0), stop=(kt == qs),
                    )
            st["ups", qb, j] = ups
            if emit_out:
                u_out(qb, j, st)

        def u_out(qb, j, st):
            qs = qb * (QBLK // P) + j
            ups = st["ups", qb, j]
            r_sb = st["r", qb]
            o_sb = outp.tile([P, O], f32, name="o_sb")
            nc.vector.tensor_scalar_mul(o_sb[:], ups[:, :O], r_sb[:, j : j + 1])
            nc.sync.dma_start(out[ds(P * qs, P), :], o_sb[:])

        def body(_iv=None):
            st = {}
            v_sb = vpool.tile([P, NKT, O], bf16, name="v_sb")
            for sb in range(NQB):
                proj(sb, v_sb, st)
                if loop_phase == "proj":
                    continue
                inter = ()
                if sb >= 1:
                    finish_z(sb - 1, st)
                    inter = [
                        (lambda j: lambda: u_chain(sb - 1, j, v_sb, st))(j)
                        for j in range(QBLK // P)
                    ]
                scores(sb, st, inter)
            if loop_phase != "proj":
                # tail: U(3) j=0,1 first (need only already-exp'd k-tiles),
                # z once the ACT backlog drains, then the rest.
                qb = NQB - 1
                u_chain(qb, 0, v_sb, st, emit_out=False)
                u_chain(qb, 1, v_sb, st, emit_out=False)
                finish_z(qb, st)
                u_out(qb, 0, st)
                u_out(qb, 1, st)
                u_chain(qb, 2, v_sb, st)
                u_chain(qb, 3, v_sb, st)

        if reps == 1:
            body()
        else:
            with tc.For_i(0, reps, 1, hint_engines=(mybir.EngineType.PE,)) as iv:
                body(iv)

    nc.compile()
    return nc


def _get_nc(s=S, reps=1, loop_phase="all"):
    key = (s, reps, loop_phase)
    if key not in _CACHE:
        _CACHE[key] = _build_nc(s, reps, loop_phase)
    return _CACHE[key]


def make_mask(qblk=512):
    kp = np.arange(P)[:, None]
    qf = np.arange(qblk)[None, :]
    m = np.stack([(qf >= P * i + kp) for i in range(4)], axis=0)
    return m.astype(ml_dtypes.bfloat16)


def make_in_maps(x, Wq, bq, Wk, bk, Wv, bv, s=S):
    bf = ml_dtypes.bfloat16
    f8 = ml_dtypes.float8_e4m3
    x, Wq, bq, Wk, bk, Wv, bv = (
        np.asarray(a, dtype=np.float32) for a in (x, Wq, bq, Wk, bk, Wv, bv)
    )
    M = (Wq.astype(np.float64) @ Wk.T.astype(np.float64)).astype(np.float32)
    wc = (Wk @ bq).astype(np.float32)
    m8 = np.ascontiguousarray(np.clip(32.0 * M, -240, 240).astype(f8))
    wv_b = np.ascontiguousarray(Wv.astype(bf))
    wcp = np.ascontiguousarray((32.0 * wc).reshape(D // P, P).T.astype(np.float32))
    bv_rep = np.ascontiguousarray(np.broadcast_to(bv, (P, O)))
    mask = make_mask()
    in_maps = []
    for b in range(x.shape[0]):
        xT_b = np.ascontiguousarray(x[b].T.astype(bf))
        x8_b = np.ascontiguousarray(np.clip(x[b].T, -240, 240).astype(f8))
        in_maps.append(
            dict(xT=xT_b, x8=x8_b, m8=m8, wv=wv_b, wcp=wcp, bv_rep=bv_rep, mask=mask)
        )
    return in_maps


def kernel(x, Wq, bq, Wk, bk, Wv, bv):
    from concourse.bass_utils import run_bass_kernel_spmd

    x = np.asarray(x, dtype=np.float32)
    assert x.shape == (B, S, D), x.shape
    nc = _get_nc(S)
    in_maps = make_in_maps(x, Wq, bq, Wk, bk, Wv, bv)
    res = run_bass_kernel_spmd(nc, in_maps, core_ids=list(range(N_CORES)))
    return np.stack([res.results[c]["out"] for c in range(N_CORES)], axis=0)
